# revision 42
# baseline (speedup 1.0000x reference)
"""Causal self-attention (B=2, T=2048, C=1024, H=16) on 8 TRN2 NeuronCores.

Sharding: 2 batches x 4 head-groups (4 heads each). Each core computes
qkv projection for its batch/head-slice, causal attention, and a partial
c_proj contribution; the host sums partials and adds b_proj.

Device layout (per core):
  xT  [C=1024, T=2048]  x[b] transposed, bf16 (host-side)
  wT  [C=1024, 768]     w_attn rows for this head slice, transposed, bf16:
                        cols [0:256]=Q feats, [256:512]=K, [512:768]=V
  bqkv [768]            matching bias slice (f32)
  wpT [256, 1024]       w_proj columns for this head slice, transposed, bf16
  out yT [1024, 2048]   partial (c_proj output)^T bf16, host-summed (f32)
                        over the 4 cores of the batch

All matmuls are bf16 (full-rate, and keeps the PE out of the fp32-pipeline
HAM throttle that caps the clock at ~1.2 GHz; bf16 activity lets it boost
to 2.4 GHz). PSUM accumulation is f32 throughout, so precision loss is
just input rounding (~1e-3 rel err overall vs the 2e-2 gate).

Causal structure: scores computed in S^T [key, query] orientation per
128-key-block x 512-query-chunk rectangles, only blocks intersecting the
causal triangle. Score matmuls for the two heads of a pair run CONCURRENTLY
on PE row-quadrants (contraction 64: rows 0-63 / 64-127, confirmed
overlapping in the HW trace). Diagonal 128x128 sub-blocks are trimmed:
scores/exp/AV only touch queries >= block start, and a single [128,128]
staircase mask (i<=j) handles the in-block triangle.
Softmax denominators come free from a ones-column appended to V (row 64
of the AV psum accumulator).
"""
import sys
import contextlib

sys.path.insert(0, "/opt/trn_rl_repo")

import numpy as np
import ml_dtypes

import concourse.bass as bass
import concourse.mybir as mybir
import concourse.tile as tile
from concourse import bacc
from concourse.bass_utils import run_bass_kernel_spmd

B, T, C, H = 2, 2048, 1024, 16
HD = 64
N_CORES = 8
HPC = 4          # heads per core
FPC = HPC * HD   # features per core = 256
QCH = 512        # query chunk
NQC = T // QCH   # 4
NCC = C // 128   # 8 contraction chunks
NTC = T // 512   # 4 token chunks

F32 = mybir.dt.float32
BF16 = mybir.dt.bfloat16

_CACHE: dict = {}


def _build():
    if "nc" in _CACHE:
        return _CACHE["nc"]
    nc = bacc.Bacc(None, target_bir_lowering=False, debug=False)

    xT_d = nc.dram_tensor("xT", [C, T], BF16, kind="ExternalInput").ap()
    wT_d = nc.dram_tensor("wT", [C, 3 * FPC], BF16, kind="ExternalInput").ap()
    bqkv_d = nc.dram_tensor("bqkv", [3 * FPC], F32, kind="ExternalInput").ap()
    wpT_d = nc.dram_tensor("wpT", [FPC, C], BF16, kind="ExternalInput").ap()
    yT_d = nc.dram_tensor("yT", [C, T], BF16, kind="ExternalOutput").ap()

    # single staircase mask for diagonal 128x128 blocks ([key, query]):
    # mask[i, j] = 1 if i <= j
    i_idx = np.arange(128)[:, None]
    j_idx = np.arange(128)[None, :]
    mask_np = (i_idx <= j_idx).astype(np.float32).astype(ml_dtypes.bfloat16)
    mask_d = nc.inline_tensor(mask_np, name="mask").ap()

    with tile.TileContext(nc) as tc:
        with contextlib.ExitStack() as ctx:
            consts = ctx.enter_context(tc.tile_pool(name="consts", bufs=1))
            xpool = ctx.enter_context(tc.tile_pool(name="x", bufs=1))
            qkpool = ctx.enter_context(tc.tile_pool(name="qk", bufs=1))
            vpool = ctx.enter_context(tc.tile_pool(name="v", bufs=1))
            ppool = ctx.enter_context(tc.tile_pool(name="p", bufs=2))
            ypool = ctx.enter_context(tc.tile_pool(name="y", bufs=3))
            opool = ctx.enter_context(tc.tile_pool(name="o", bufs=3))
            rpool = ctx.enter_context(tc.tile_pool(name="r", bufs=2))
            big_ps = ctx.enter_context(tc.tile_pool(name="big_ps", bufs=2, space="PSUM"))
            s_ps = ctx.enter_context(tc.tile_pool(name="s_ps", bufs=2, space="PSUM"))
            y_ps = ctx.enter_context(tc.tile_pool(name="y_ps", bufs=2, space="PSUM"))

            # ---- constants / weights. Input DMAs are spread across the four
            # HWDGE-capable queues so descriptor generation doesn't serialize
            # on one sequencer; the tc=0 slices of x land first so the first
            # qkv accumulation group can start immediately. ----
            queues = [nc.sync, nc.scalar]
            # DMA queues serialize on transfer completion, so: tiny consts
            # first on sync (they gate the first qkv bias-add), then the
            # critical x slices on sync while the weights go on scalar.
            qk_bias_t = consts.tile([128, 4, 1], F32, tag="qkb", name="qkb")
            queues[0].dma_start(
                out=qk_bias_t[:],
                in_=bqkv_d[0:512].rearrange("(j p) -> p j", j=4).unsqueeze(-1),
            )
            qk_bias = {fb: qk_bias_t[:, fb, :] for fb in range(4)}
            v_bias = consts.tile([128, FPC], F32, tag="vbias")
            queues[0].dma_start(
                out=v_bias[:], in_=bqkv_d[512:768].partition_broadcast(128)
            )
            mask = consts.tile([128, 128], BF16, tag="mask")
            queues[0].dma_start(out=mask[:], in_=mask_d[:])
            # cc-paired tiles: wtp[ccp][:, j, :] is contraction chunk 2*ccp+j.
            # Pairing halves the DMA instruction count.
            wtp = {}
            xtp = {}
            for ccp in range(NCC // 2):
                rows = slice(ccp * 256, (ccp + 1) * 256)
                wtp[ccp] = consts.tile([128, 2, 3 * FPC], BF16, tag=f"w{ccp}", name=f"w{ccp}")
                queues[1].dma_start(
                    out=wtp[ccp][:],
                    in_=wT_d[rows, :].rearrange("(j p) f -> p j f", j=2),
                )
                xtp[ccp] = xpool.tile([128, 2, T], BF16, tag=f"x{ccp}", name=f"x{ccp}")
                queues[0].dma_start(
                    out=xtp[ccp][:, :, 0:512],
                    in_=xT_d[rows, 0:512].rearrange("(j p) t -> p j t", j=2),
                )
            for ccp in range(NCC // 2):
                rows = slice(ccp * 256, (ccp + 1) * 256)
                queues[ccp % 2].dma_start(
                    out=xtp[ccp][:, :, 512:T],
                    in_=xT_d[rows, 512:T].rearrange("(j p) t -> p j t", j=2),
                )
            # per-chunk views: wt[cc] [128, 768], xt[cc] [128, T]
            wt = {}
            xt = {}
            for cc in range(NCC):
                wt[cc] = wtp[cc // 2][:, cc % 2, :]
                xt[cc] = xtp[cc // 2][:, cc % 2, :]
            wp_t = consts.tile([128, 2, C], BF16, tag="wp", name="wp")
            queues[0].dma_start(
                out=wp_t[:], in_=wpT_d[:].rearrange("(j p) f -> p j f", j=2)
            )
            wp = {hc: wp_t[:, hc, :] for hc in range(2)}

            # ---- QKV projection ----
            # feature-major Q^T, K^T: qk[(fb, tc)] [128, 512], fb 0..1 = Q
            # (heads 0-1, 2-3), fb 2..3 = K
            qk = {}
            vt_by_tb = {}

            def qkv_groups(tc_i):
                """Generator: one PE accumulation group per next() (8 total)."""
                tok = slice(tc_i * 512, (tc_i + 1) * 512)
                for fb in range(4):
                    ps = big_ps.tile([128, 512], F32, tag="bigps", name=f"qps{tc_i}_{fb}")
                    for cc in range(NCC):
                        nc.tensor.matmul(
                            ps[:],
                            wt[cc][:, fb * 128:(fb + 1) * 128],
                            xt[cc][:, tok],
                            start=(cc == 0),
                            stop=(cc == NCC - 1),
                        )
                    qk[(fb, tc_i)] = qkpool.tile([128, 512], BF16, tag=f"qk{fb}_{tc_i}", name=f"qk{fb}_{tc_i}")
                    nc.vector.tensor_scalar_add(qk[(fb, tc_i)][:], ps[:], qk_bias[fb][:])
                    yield
                # token-major V_ext tiles [128 tokens, 4 heads, 66] (64 V cols,
                # col 64 = ones for the softmax denominator, col 65 pad)
                for tb in range(tc_i * 4, tc_i * 4 + 4):
                    ps = big_ps.tile([128, FPC], F32, tag="bigps", name=f"vps{tb}")
                    for cc in range(NCC):
                        nc.tensor.matmul(
                            ps[:],
                            xt[cc][:, tb * 128:(tb + 1) * 128],
                            wt[cc][:, 512:768],
                            start=(cc == 0),
                            stop=(cc == NCC - 1),
                        )
                    vt = vpool.tile([128, HPC, 66], BF16, tag=f"v{tb}")
                    nc.vector.tensor_add(
                        vt[:, :, 0:64],
                        ps[:].rearrange("p (h d) -> p h d", h=HPC),
                        v_bias[:].rearrange("p (h d) -> p h d", h=HPC),
                    )
                    nc.vector.memset(vt[:, :, 64:65], 1.0)
                    vt_by_tb[tb] = vt
                    yield

            yT_by_qc = {}

            def emit_attn(qc, fillers=(), n_fill=0, fill_from=0, after_hp0=None):
                yT_pair = {}
                for hc in range(2):
                    yT_pair[hc] = ypool.tile([128, 512], BF16, tag=f"yp{hc}", name=f"yp{hc}_{qc}")
                yT_by_qc[qc] = yT_pair
                kmax = 4 * (qc + 1)
                # interleave filler PE groups (next chunk's qkv, prev chunk's
                # c_proj) into the kb loop: the PE queue is in-order, so
                # without ready filler work behind each AV matmul the PE
                # stalls whenever exp (ACT) is the pacer.
                fillers = [f for f in fillers if f is not None]
                slots = max(1, 2 * kmax * 2 - fill_from)
                stride = max(1, slots // n_fill) if n_fill else slots + 1
                slot = [0]

                def fill_one():
                    while fillers:
                        try:
                            next(fillers[0])
                            fillers.append(fillers.pop(0))
                            return
                        except StopIteration:
                            fillers.pop(0)

                def fill_point():
                    slot[0] += 1
                    if slot[0] <= fill_from or (slot[0] - fill_from) % stride:
                        return
                    fill_one()

                for hp in range(2):
                    # scores: both heads of the pair run concurrently on PE
                    # row-quadrants (contraction rows 0-63 / 64-127)
                    ys = {hb: y_ps.tile([65, 512], F32, tag="yps", name=f"yps{qc}_{hp}_{hb}") for hb in range(2)}
                    pt_by_kb = {}

                    def emit_av(kb):
                        q_lo = max(kb * 128 - qc * 512, 0)
                        for hb in range(2):
                            nc.tensor.matmul(
                                ys[hb][:, q_lo:512],
                                vt_by_tb[kb][:, 2 * hp + hb, 0:65],
                                pt_by_kb[kb][:, hb, q_lo:512],
                                start=(kb == 0),
                                stop=(kb == kmax - 1),
                            )

                    # AV for kb is emitted after scores for kb+1: the next
                    # score pair (plus filler) hides exp(kb)'s latency so the
                    # AV rarely stalls the in-order PE queue.
                    for kb in range(kmax):
                        g = kb * 128 - qc * 512  # diag offset; >=0 on diagonal blocks
                        q_lo = max(g, 0)
                        sp = s_ps.tile([128, 1024], F32, tag="sps")
                        for hb in range(2):
                            rows = slice(hb * 64, hb * 64 + 64)
                            nc.tensor.matmul(
                                sp[:, hb * 512 + q_lo:(hb + 1) * 512],
                                qk[(2 + hp, kb // 4)][rows, (kb % 4) * 128:(kb % 4 + 1) * 128],
                                qk[(hp, qc)][rows, q_lo:512],
                                start=True,
                                stop=True,
                            )
                        fill_point()
                        pt = ppool.tile([128, 2, 512], BF16, tag=f"p{kb % 6}", name=f"p{qc}_{hp}_{kb}")
                        pt_by_kb[kb] = pt
                        # one exp instruction covers both heads (strided AP)
                        nc.scalar.activation(
                            pt[:, :, q_lo:512],
                            sp[:].rearrange("p (h q) -> p h q", h=2)[:, :, q_lo:512],
                            mybir.ActivationFunctionType.Exp,
                            scale=0.125,
                        )
                        if g >= 0:
                            # in-block staircase on the single straddling
                            # 128-query block
                            for hb in range(2):
                                nc.vector.tensor_mul(
                                    pt[:, hb, g:g + 128],
                                    pt[:, hb, g:g + 128],
                                    mask[:],
                                )
                        if kb > 0:
                            emit_av(kb - 1)
                        fill_point()
                    emit_av(kmax - 1)
                    # pull extra filler work to cover the normalize chain
                    # latency before the next head-pair's first AV (WAR on
                    # the ys psum bufs)
                    fill_one()
                    fill_one()
                    fill_one()
                    for hb in range(2):
                        yp = ys[hb]
                        denom = rpool.tile([1, 512], F32, tag="denom")
                        nc.scalar.copy(denom[:], yp[64:65, :])
                        recip = rpool.tile([1, 512], F32, tag="recip")
                        nc.vector.reciprocal_approx_fast(out=recip[:], in_=denom[:])
                        recip_b = rpool.tile([64, 512], F32, tag="recip_b")
                        nc.gpsimd.partition_broadcast(recip_b[:], recip[:])
                        nc.vector.tensor_mul(
                            yT_pair[hp][hb * 64:(hb + 1) * 64, :], yp[0:64, :], recip_b[:]
                        )
                    if hp == 0 and after_hp0 is not None:
                        fillers.append(after_hp0())
                # leftovers (e.g. odd group counts)
                while fillers:
                    try:
                        next(fillers[0])
                        fillers.append(fillers.pop(0))
                    except StopIteration:
                        fillers.pop(0)

            def cproj_groups(qc):
                """Generator: one c_proj ob-pair group per next() (4 total).
                Output is bf16 (halves DMA bytes; host sums partials in f32),
                with two 128-row blocks packed per DMA."""
                yT_pair = yT_by_qc[qc]
                for obp in range(4):
                    ots = opool.tile([128, 2, 512], BF16, tag="ot", name=f"ot{qc}_{obp}")
                    for j in range(2):
                        ob = 2 * obp + j
                        op = big_ps.tile([128, 512], F32, tag="bigps", name=f"ops{qc}_{ob}")
                        for hc in range(2):
                            nc.tensor.matmul(
                                op[:],
                                wp[hc][:, ob * 128:(ob + 1) * 128],
                                yT_pair[hc][:],
                                start=(hc == 0),
                                stop=(hc == 1),
                            )
                        if j == 0:
                            nc.vector.tensor_copy(ots[:, j, :], op[:])
                        else:
                            nc.scalar.copy(ots[:, j, :], op[:])
                    queues[obp % 2].dma_start(
                        out=yT_d[obp * 256:(obp + 1) * 256, qc * 512:(qc + 1) * 512]
                        .rearrange("(j p) t -> p j t", j=2),
                        in_=ots[:],
                    )
                    yield

            # software pipeline: attn(qc)'s kb loop is interleaved with the
            # next chunk's qkv groups and the previous chunk's c_proj groups,
            # so the in-order PE queue always has ready work behind an AV
            # matmul that is waiting on exp.
            for _ in qkv_groups(0):
                pass
            # attn0's fillers start late so they don't head-of-line block on
            # the xT tail DMAs still in flight
            emit_attn(0, [qkv_groups(1)], n_fill=8, fill_from=6)
            emit_attn(1, [qkv_groups(2), cproj_groups(0)], n_fill=12)
            emit_attn(2, [qkv_groups(3)], n_fill=8)
            emit_attn(3, [cproj_groups(1), cproj_groups(2)], n_fill=8)
            # tail-chunk c_proj, split by head-pair half: the hc=0 matmuls are
            # ready as soon as attn3 returns (yT_pair[0] was normalized before
            # hp1's kb loop), so they fill the PE gap while hp1's normalize
            # chain completes; only the hc=1 half plus a merge-add trails it.
            yT3 = yT_by_qc[3]
            cp3 = {}
            for obp in range(4):
                t = opool.tile([128, 2, 512], F32, tag="ot0", name=f"ot0_{obp}", bufs=4)
                cp3[obp] = t
                for j in range(2):
                    ob = 2 * obp + j
                    op = big_ps.tile([128, 512], F32, tag="bigps", name=f"o3a{ob}")
                    nc.tensor.matmul(
                        op[:], wp[0][:, ob * 128:(ob + 1) * 128],
                        yT3[0][:], start=True, stop=True,
                    )
                    if j == 0:
                        nc.vector.tensor_copy(t[:, j, :], op[:])
                    else:
                        nc.scalar.copy(t[:, j, :], op[:])
            for obp in range(4):
                ots = opool.tile([128, 2, 512], BF16, tag="ot", name=f"ot3_{obp}")
                for j in range(2):
                    ob = 2 * obp + j
                    op = big_ps.tile([128, 512], F32, tag="bigps", name=f"o3b{ob}")
                    nc.tensor.matmul(
                        op[:], wp[1][:, ob * 128:(ob + 1) * 128],
                        yT3[1][:], start=True, stop=True,
                    )
                    nc.vector.tensor_add(ots[:, j, :], op[:], cp3[obp][:, j, :])
                queues[obp % 2].dma_start(
                    out=yT_d[obp * 256:(obp + 1) * 256, 3 * 512:4 * 512]
                    .rearrange("(j p) t -> p j t", j=2),
                    in_=ots[:],
                )
    nc.compile()
    _CACHE["nc"] = nc
    return nc


def _make_in_maps(x, w_attn, b_attn, w_proj):
    bf16 = ml_dtypes.bfloat16
    in_maps = []
    for core in range(N_CORES):
        b, s = core // 4, core % 4
        f0 = FPC * s
        xT = np.ascontiguousarray(x[b].T).astype(bf16)
        wT = np.ascontiguousarray(
            np.concatenate(
                [
                    w_attn[f0:f0 + FPC],
                    w_attn[C + f0:C + f0 + FPC],
                    w_attn[2 * C + f0:2 * C + f0 + FPC],
                ],
                axis=0,
            ).T
        ).astype(bf16)
        bqkv = np.ascontiguousarray(
            np.concatenate(
                [
                    b_attn[f0:f0 + FPC],
                    b_attn[C + f0:C + f0 + FPC],
                    b_attn[2 * C + f0:2 * C + f0 + FPC],
                ]
            )
        ).astype(np.float32)
        wpT = np.ascontiguousarray(w_proj[:, f0:f0 + FPC].T).astype(bf16)
        in_maps.append({"xT": xT, "wT": wT, "bqkv": bqkv, "wpT": wpT})
    return in_maps


def kernel(x, w_attn, b_attn, w_proj, b_proj):
    nc = _build()
    in_maps = _make_in_maps(x, w_attn, b_attn, w_proj)
    _CACHE["in_maps"] = in_maps

    res = run_bass_kernel_spmd(nc, in_maps, list(range(N_CORES)))
    out = np.empty((B, T, C), dtype=np.float32)
    for b in range(B):
        acc = res.results[4 * b]["yT"].astype(np.float32)
        for s in range(1, 4):
            acc = acc + res.results[4 * b + s]["yT"]
        out[b] = acc.T + b_proj[None, :]
    return out
